# revision 58
# baseline (speedup 1.0000x reference)
"""Causal self-attention (B=4, T=2048, D=1024, H=16) on 8 trn2 NeuronCores.

Sharding: core c handles batch b=c//2 and head-group g=c%2 (8 heads, 512
features). Each core computes q/k/v projections for its feature slice, causal
attention for its 8 heads, and a partial output projection (row-parallel W_o).
The host sums the two partial outputs per batch and adds b_o.

All matmuls run in bf16 (inputs cast on host; fp32 psum accumulation).
Design notes (cost model: matmul cost = out free size x 1 cyc/row @2.4GHz):
- P@V is computed as out[q_block(128), Dh+1] so each matmul pays only 65
  columns instead of 512 -> P@V costs 8840 cols/head vs 17408.  The ones
  column appended to v gives the softmax denominator in the same matmuls.
- y comes out [q, f]; a PE transpose (128x128 bf16 blocks via identity)
  restores y^T[f, q] for the row-parallel output projection.
- softmax exp runs on the Activation engine (the #2 engine at ~150us);
  projections are software-pipelined into the attention chunks so the PE
  never waits on exp.
- per-(q-block) denominators land in psum cols [*, b, 64]; one DVE
  reciprocal + one broadcast multiply per (chunk, head) normalizes y.
"""
import sys

sys.path.insert(0, "/opt/trn_rl_repo")

import numpy as np
import ml_dtypes

import concourse.bacc as bacc
import concourse.mybir as mybir
from concourse.tile import TileContext
from concourse.bass_utils import run_bass_kernel_spmd

B, T, D, H = 4, 2048, 1024, 16
Dh = D // H                    # 64
NCORES = 8
F = D // 2                     # 512 features (8 heads) per core
KD = D // 128                  # 8 contraction tiles for projections
PAIRS = F // 128               # 4 head-pair feature tiles
NKT = T // 128                 # 16 key/value 128-blocks
NTC = T // 512                 # 4 query chunks of 512
HL = H // 2                    # 8 local heads

F32 = mybir.dt.float32
BF16 = mybir.dt.bfloat16
EXP = mybir.ActivationFunctionType.Exp

_NC_CACHE = None


def build_nc():
    nc = bacc.Bacc(None, target_bir_lowering=False, debug=False)

    xT = nc.dram_tensor("xT", [D, T], BF16, kind="ExternalInput")
    wqT = nc.dram_tensor("wqT", [D, F], BF16, kind="ExternalInput")
    wkT = nc.dram_tensor("wkT", [D, F], BF16, kind="ExternalInput")
    wvT = nc.dram_tensor("wvT", [D, F], BF16, kind="ExternalInput")
    woT = nc.dram_tensor("woT", [F, D], BF16, kind="ExternalInput")
    tri = nc.dram_tensor("tri", [128, 128], BF16, kind="ExternalInput")
    ident = nc.dram_tensor("ident", [128, 128], BF16, kind="ExternalInput")
    outT = nc.dram_tensor("outT", [D, T], BF16, kind="ExternalOutput")

    with TileContext(nc) as tc:
        with (
            tc.tile_pool(name="persist", bufs=1) as persist,
            tc.tile_pool(name="ptpool", bufs=22) as ptpool,
            tc.tile_pool(name="ybpool", bufs=2) as ybpool,
            tc.tile_pool(name="recpool", bufs=4) as recpool,
            tc.tile_pool(name="ostage", bufs=4) as ostage,
            tc.tile_pool(name="pjo", bufs=2, space="PSUM") as pjo,
            tc.tile_pool(name="sps", bufs=2, space="PSUM") as sps,
        ):
            xt = persist.tile([128, KD, T], BF16)
            kTt = persist.tile([128, PAIRS, T], BF16)
            qTt = persist.tile([128, PAIRS, T], BF16)
            vo = persist.tile([128, NKT, HL, Dh + 1], BF16)
            yTt = persist.tile([128, PAIRS, T], BF16)
            wo = persist.tile([128, PAIRS, D], BF16)
            wkt = persist.tile([128, KD, F], BF16)
            wqt = persist.tile([128, KD, F], BF16)
            wvt = persist.tile([128, KD, F], BF16)
            trit = persist.tile([128, 128], BF16)
            idt = persist.tile([128, 128], BF16)
            onesc = persist.tile([128, 1], BF16)

            nc.vector.memset(onesc[:], 1.0)
            nc.vector.tensor_copy(
                vo[:, :, :, Dh : Dh + 1], onesc.broadcast_to([128, NKT, HL, 1])
            )

            xTr = xT.rearrange("(k p) t -> p k t", p=128)
            wkr = wkT.rearrange("(k p) f -> p k f", p=128)
            wqr = wqT.rearrange("(k p) f -> p k f", p=128)
            # DMA order follows the consumption order of the software
            # pipeline: just enough of wk/x/wq to start k(0,0)/q(0,0), then
            # the rest in first-use order (transfers serialize on the DMA
            # engines, so order = arrival time).
            nc.sync.dma_start(wkt[:, :, 0:128], wkr[:, :, 0:128])
            nc.sync.dma_start(xt[:, 0:2, 0:512], xTr[:, 0:2, 0:512])
            nc.sync.dma_start(xt[:, 2:4, 0:512], xTr[:, 2:4, 0:512])
            nc.sync.dma_start(wqt[:, :, 0:128], wqr[:, :, 0:128])
            nc.sync.dma_start(xt[:, 4:6, 0:512], xTr[:, 4:6, 0:512])
            nc.sync.dma_start(xt[:, 6:8, 0:512], xTr[:, 6:8, 0:512])
            nc.sync.dma_start(wkt[:, :, 128:512], wkr[:, :, 128:512])
            nc.sync.dma_start(wqt[:, :, 128:512], wqr[:, :, 128:512])
            nc.sync.dma_start(tri_:=trit[:], tri[:])
            nc.sync.dma_start(wvt[:], wvT.rearrange("(k p) f -> p k f", p=128))
            nc.sync.dma_start(idt[:], ident[:])
            nc.sync.dma_start(xt[:, :, 512:1024], xTr[:, :, 512:1024])
            nc.sync.dma_start(xt[:, :, 1024:1536], xTr[:, :, 1024:1536])
            nc.sync.dma_start(xt[:, :, 1536:2048], xTr[:, :, 1536:2048])
            nc.sync.dma_start(wo[:], woT.rearrange("(k p) m -> p k m", p=128))

            def proj_qk(wt, dst, c, t):
                """dst[:, t, 512c:512c+512] = (W^T x)[128 f-rows of pair t]."""
                ps = pjo.tile([128, 512], F32, name="qk_ps", tag="pjo")
                for k in range(KD):
                    nc.tensor.matmul(
                        ps[:],
                        wt[:, k, 128 * t : 128 * t + 128],
                        xt[:, k, 512 * c : 512 * c + 512],
                        start=(k == 0),
                        stop=(k == KD - 1),
                    )
                nc.vector.tensor_copy(dst[:, t, 512 * c : 512 * c + 512], ps[:])

            def proj_v(tb):
                """vo[:, tb, :, 0:64] = x[128 toks of tb] @ W_v (head-major)."""
                ps = pjo.tile([128, 512], F32, name="v_ps", tag="pjo")
                # one accumulation group at a time per psum bank
                for q4 in range(2):
                    for k in range(KD):
                        nc.tensor.matmul(
                            ps[:, 256 * q4 : 256 * q4 + 256],
                            xt[:, k, 128 * tb : 128 * tb + 128],
                            wvt[:, k, 256 * q4 : 256 * q4 + 256],
                            start=(k == 0),
                            stop=(k == KD - 1),
                        )
                nc.vector.tensor_copy(
                    vo[:, tb, :, 0:Dh], ps.rearrange("p (h d) -> p h d", d=Dh)
                )

            def s_group(j, h, kbs, pts):
                """Scores for up to 3 full key blocks OR the 4 diagonal
                blocks of chunk j, gap-free-packed into one [128, 1536]
                psum tile (each block inside a single 512-col bank) so a
                single exp instruction covers the group.  pts[h][kb]
                records (pt_tile, base - col0) for pv_q's addressing.
                """
                t, s = h // 2, h % 2
                rows = slice(64 * s, 64 * s + 64)
                ps = sps.tile([128, 1536], F32, name="s_ps")
                base = 0
                placements = []
                for kb in kbs:
                    d = kb - 4 * j
                    col0 = 128 * d if d > 0 else 0
                    placements.append((kb, base, col0))
                    base += 512 - col0
                total = base
                for kb, bb, col0 in placements:
                    nc.tensor.matmul(
                        ps[:, bb : bb + 512 - col0],
                        kTt[rows, t, 128 * kb : 128 * kb + 128],
                        qTt[rows, t, 512 * j + col0 : 512 * j + 512],
                        start=True,
                        stop=True,
                    )
                pt = ptpool.tile([128, 1536], BF16, name="pt")
                nc.scalar.activation(
                    pt[:, 0:total], ps[:, 0:total], EXP,
                    scale=float(Dh) ** -0.5,
                )
                for kb, bb, col0 in placements:
                    if kb >= 4 * j:
                        # on gpsimd: keeps the DVE queue free of exp-waiting
                        # work (recip/norm must not queue behind masks)
                        nc.gpsimd.tensor_mul(
                            pt[:, bb : bb + 128],
                            pt[:, bb : bb + 128],
                            trit[:],
                        )
                    pts.setdefault(h, {})[kb] = (pt, bb - col0)

            def s_unit(j, h, pts):
                full = list(range(4 * j))
                for i in range(0, len(full), 3):
                    s_group(j, h, full[i : i + 3], pts)
                # diagonal quad ordered so widths (512,384,128,256) pack
                # the banks as 512 | 384+128 | 256: gap- and crossing-free
                s_group(j, h, [4 * j, 4 * j + 1, 4 * j + 3, 4 * j + 2], pts)

            def pv_q(j, h, b, pts, ypss):
                """P@V accumulation for q-block b of (chunk j, head h)."""
                if h not in ypss:
                    ypss[h] = pjo.tile(
                        [128, 4, Dh + 1], F32, name="y_ps", tag="pjo"
                    )
                yps = ypss[h]
                qb = 4 * j + b
                for kb in range(qb + 1):
                    pt, off0 = pts[h][kb]
                    nc.tensor.matmul(
                        yps[:, b, :],
                        pt[:, off0 + 128 * b : off0 + 128 * b + 128],
                        vo[:, kb, h, :],
                        start=(kb == 0),
                        stop=(kb == qb),
                    )

            def pv_fin(j, h, ypss, yb):
                """reciprocal of denominators + normalized write to yb."""
                yps = ypss.pop(h)
                rec = recpool.tile([128, 4, 1], F32, name="rec")
                nc.vector.reciprocal(rec[:], yps[:, :, Dh : Dh + 1])
                nc.vector.tensor_mul(
                    yb[:, :, Dh * h : Dh * h + Dh],
                    yps[:, :, 0:Dh],
                    rec.broadcast_to([128, 4, Dh]),
                )

            def pv_unit(j, h, pts, yb, ypss=None):
                if ypss is None:
                    ypss = {}
                for b in range(4):
                    pv_q(j, h, b, pts, ypss)
                pv_fin(j, h, ypss, yb)

            def transpose_ft(j, ft, yb):
                """yTt[:, ft, chunk j] = yb[:, :, ftile].T (bf16 PE blocks).

                Depends only on heads 2ft and 2ft+1, so it can run right
                after their PV units instead of at the chunk tail.
                """
                tps = pjo.tile([128, 4, 128], BF16, name="t_ps", tag="pjo")
                for b in range(4):
                    nc.tensor.matmul(
                        tps[:, b, :],
                        yb[:, b, 128 * ft : 128 * ft + 128],
                        idt[:],
                        is_transpose=True,
                    )
                nc.vector.tensor_copy(
                    yTt[:, ft, 512 * j : 512 * j + 512],
                    tps.rearrange("p b q -> p (b q)"),
                )

            def oproj(j, m):
                ps = pjo.tile([128, 512], F32, name="o_ps", tag="pjo")
                for kf in range(PAIRS):
                    nc.tensor.matmul(
                        ps[:],
                        wo[:, kf, 128 * m : 128 * m + 128],
                        yTt[:, kf, 512 * j : 512 * j + 512],
                        start=(kf == 0),
                        stop=(kf == PAIRS - 1),
                    )
                st = ostage.tile([128, 512], BF16, name="o_st", tag="o_st")
                nc.vector.tensor_copy(st[:], ps[:])
                nc.sync.dma_start(
                    outT[128 * m : 128 * m + 128, 512 * j : 512 * j + 512],
                    st[:],
                )

            # ---- schedule ----------------------------------------------
            # Per chunk j: S (scores+exp+mask) and P (P@V+normalize) units
            # pipelined with skew 2 so exp(h) overlaps later matmuls, and
            # projection/oproj filler woven between units to keep the PE
            # busy while the Activation engine chews on exp.
            def chunk_stream(j, fillers, pts=None, first_s=0, extra_s=(), back=False):
                """Unit steps with fillers interleaved evenly (chunks >= 1).

                pts/first_s: S units of this chunk already issued by the
                previous chunk's stream (pulled heads).  extra_s: S units of
                the NEXT chunk to issue at this chunk's tail, as
                [(h, pts_next)] entries.
                """
                if pts is None:
                    pts = {}
                yb = ybpool.tile([128, 4, F], BF16, name="yb", tag="yb")
                order = []
                for h in range(first_s, 8):
                    if h >= 2:
                        order.append(("p", h - 2))
                        if (h - 2) % 2 == 1:
                            order.append(("t", (h - 2) // 2))
                    order.append(("s", h))
                order += [("p", 6), ("p", 7), ("t", 3)]
                steps = []
                for kind, h in order:
                    if kind == "s":
                        steps.append(lambda h=h: s_unit(j, h, pts))
                    elif kind == "p":
                        steps.append(lambda h=h: pv_unit(j, h, pts, yb))
                    else:
                        steps.append(lambda h=h: transpose_ft(j, h, yb))
                for (h_nxt, pts_nxt) in extra_s:
                    steps.append(
                        lambda h=h_nxt, d=pts_nxt: s_unit(j + 1, h, d)
                    )
                # weave fillers between steps
                nf, ns = len(fillers), len(steps)
                out = []
                fi = 0
                for si, st_ in enumerate(steps):
                    out.append(st_)
                    if back:
                        # back-loaded: reserve fillers for the tail units
                        want = int(round(nf * ((si + 1) / ns) ** 2))
                    else:
                        want = int(round((si + 1) * nf / ns))
                    while fi < min(want, nf):
                        out.append(fillers[fi])
                        fi += 1
                while fi < nf:
                    out.append(fillers[fi])
                    fi += 1
                return out

            # ---- chunk 0 (hand-ordered) --------------------------------
            # k/q pair tiles and v blocks are issued before the S/P units
            # that consume them; k/q of chunk 1 and v(4..7) (chunk 1's PV
            # inputs) ride along as PE filler under chunk 0's exp.
            pts0 = {}
            yb0 = ybpool.tile([128, 4, F], BF16, name="yb0", tag="yb")
            proj_qk(wkt, kTt, 0, 0)
            proj_qk(wqt, qTt, 0, 0)
            ops0 = [
                lambda: s_unit(0, 0, pts0),
                lambda: s_unit(0, 1, pts0),
                lambda: proj_qk(wkt, kTt, 0, 1),
                lambda: proj_qk(wqt, qTt, 0, 1),
                lambda: s_unit(0, 2, pts0),
                lambda: s_unit(0, 3, pts0),
                lambda: proj_qk(wkt, kTt, 0, 2),
                lambda: proj_qk(wqt, qTt, 0, 2),
                lambda: s_unit(0, 4, pts0),
                lambda: proj_v(0),
                lambda: proj_v(1),
                lambda: proj_v(2),
                lambda: proj_v(3),
                lambda: pv_unit(0, 0, pts0, yb0),
                lambda: proj_qk(wkt, kTt, 0, 3),
                lambda: proj_qk(wqt, qTt, 0, 3),
                lambda: s_unit(0, 5, pts0),
                lambda: pv_unit(0, 1, pts0, yb0),
                lambda: transpose_ft(0, 0, yb0),
                lambda: proj_qk(wkt, kTt, 1, 0),
                lambda: proj_qk(wqt, qTt, 1, 0),
                lambda: s_unit(0, 6, pts0),
                lambda: pv_unit(0, 2, pts0, yb0),
                lambda: proj_qk(wkt, kTt, 1, 1),
                lambda: proj_qk(wqt, qTt, 1, 1),
                lambda: s_unit(0, 7, pts0),
                lambda: pv_unit(0, 3, pts0, yb0),
                lambda: transpose_ft(0, 1, yb0),
                lambda: proj_qk(wkt, kTt, 1, 2),
                lambda: proj_qk(wqt, qTt, 1, 2),
                lambda: pv_unit(0, 4, pts0, yb0),
                lambda: proj_qk(wkt, kTt, 1, 3),
                lambda: proj_qk(wqt, qTt, 1, 3),
                lambda: pv_unit(0, 5, pts0, yb0),
                lambda: transpose_ft(0, 2, yb0),
                lambda: proj_v(4),
                lambda: proj_v(5),
                lambda: pv_unit(0, 6, pts0, yb0),
                lambda: proj_v(6),
                lambda: proj_v(7),
                lambda: pv_unit(0, 7, pts0, yb0),
                lambda: transpose_ft(0, 3, yb0),
            ]

            f1 = []
            for t in range(PAIRS):
                f1.append(lambda t=t: proj_qk(wkt, kTt, 2, t))
                f1.append(lambda t=t: proj_qk(wqt, qTt, 2, t))
            for tb in range(8, 12):
                f1.append(lambda tb=tb: proj_v(tb))

            # chunk 2 fillers: k/q of chunk 3 first (the pulled chunk-3 S
            # units at the end of chunk 2 need them), then v(12..15) and
            # oproj(0); the first two chunk-3 S units are woven at chunk
            # 2's tail to pre-feed the Activation engine.
            pts2 = {}
            pts3 = {}
            f2 = []
            for t in range(PAIRS):
                f2.append(lambda t=t: proj_qk(wkt, kTt, 3, t))
                f2.append(lambda t=t: proj_qk(wqt, qTt, 3, t))
            for tb in range(12, 16):
                f2.append(lambda tb=tb: proj_v(tb))
            for m in range(4):
                f2.append(lambda m=m: oproj(0, m))

            f3 = [lambda j=j, m=m: oproj(j, m)
                  for (j, m) in [(0, m) for m in range(4, 8)]
                  + [(1, m) for m in range(8)] + [(2, m) for m in range(8)]]

            for op in ops0:
                op()
            for op in chunk_stream(1, f1):
                op()
            for op in chunk_stream(2, f2, extra_s=[(0, pts3), (1, pts3)]):
                op()
            for op in chunk_stream(3, f3, pts=pts3, first_s=2):
                op()
            for m in range(8):
                oproj(3, m)

    nc.finalize()
    return nc


def make_in_maps(x, W_q, W_k, W_v, W_o):
    bf = ml_dtypes.bfloat16
    tri = np.triu(np.ones((128, 128), dtype=np.float32)).astype(bf)
    idm = np.eye(128, dtype=np.float32).astype(bf)
    in_maps = []
    for c in range(NCORES):
        b, g = c // 2, c % 2
        fs = slice(F * g, F * g + F)
        in_maps.append(
            {
                "xT": np.ascontiguousarray(x[b].T).astype(bf),
                "wqT": np.ascontiguousarray(W_q[fs, :].T).astype(bf),
                "wkT": np.ascontiguousarray(W_k[fs, :].T).astype(bf),
                "wvT": np.ascontiguousarray(W_v[fs, :].T).astype(bf),
                "woT": np.ascontiguousarray(W_o[:, fs].T).astype(bf),
                "tri": tri,
                "ident": idm,
            }
        )
    return in_maps


def kernel(x, W_q, W_k, W_v, W_o, b_o):
    global _NC_CACHE
    x = np.asarray(x, dtype=np.float32)
    W_q = np.asarray(W_q, dtype=np.float32)
    W_k = np.asarray(W_k, dtype=np.float32)
    W_v = np.asarray(W_v, dtype=np.float32)
    W_o = np.asarray(W_o, dtype=np.float32)
    b_o = np.asarray(b_o, dtype=np.float32)

    if _NC_CACHE is None:
        _NC_CACHE = build_nc()
    nc = _NC_CACHE

    in_maps = make_in_maps(x, W_q, W_k, W_v, W_o)
    res = run_bass_kernel_spmd(nc, in_maps, core_ids=list(range(NCORES)))

    out = np.empty((B, T, D), dtype=np.float32)
    for b in range(B):
        acc = res.results[2 * b]["outT"].astype(np.float32) + res.results[
            2 * b + 1
        ]["outT"].astype(np.float32)
        out[b] = acc.T + b_o
    return out


if __name__ == "__main__":
    rng = np.random.default_rng(0)
    inputs = {
        "x": rng.standard_normal((B, T, D), dtype=np.float32),
        "W_q": rng.standard_normal((D, D), dtype=np.float32) / 32,
        "W_k": rng.standard_normal((D, D), dtype=np.float32) / 32,
        "W_v": rng.standard_normal((D, D), dtype=np.float32) / 32,
        "W_o": rng.standard_normal((D, D), dtype=np.float32) / 32,
        "b_o": rng.standard_normal((D,), dtype=np.float32) * 0.02,
    }
    out = kernel(**inputs)
    print("ran ok", out.shape, out.dtype)


# revision 65
# speedup vs baseline: 1.0079x; 1.0079x over previous
"""Causal self-attention (B=4, T=2048, D=1024, H=16) on 8 trn2 NeuronCores.

Sharding: core c handles batch b=c//2 and head-group g=c%2 (8 heads, 512
features). Each core computes q/k/v projections for its feature slice, causal
attention for its 8 heads, and a partial output projection (row-parallel W_o).
The host sums the two partial outputs per batch and adds b_o.

All matmuls run in bf16 (inputs cast on host; fp32 psum accumulation).
Design notes (cost model: matmul cost = out free size x 1 cyc/row @2.4GHz):
- P@V is computed as out[q_block(128), Dh+1] so each matmul pays only 65
  columns instead of 512 -> P@V costs 8840 cols/head vs 17408.  The ones
  column appended to v gives the softmax denominator in the same matmuls.
- y comes out [q, f]; a PE transpose (128x128 bf16 blocks via identity)
  restores y^T[f, q] for the row-parallel output projection.
- softmax exp runs on the Activation engine (the #2 engine at ~150us);
  projections are software-pipelined into the attention chunks so the PE
  never waits on exp.
- per-(q-block) denominators land in psum cols [*, b, 64]; one DVE
  reciprocal + one broadcast multiply per (chunk, head) normalizes y.
"""
import sys

sys.path.insert(0, "/opt/trn_rl_repo")

import numpy as np
import ml_dtypes

import concourse.bacc as bacc
import concourse.mybir as mybir
from concourse.tile import TileContext
from concourse.bass_utils import run_bass_kernel_spmd

B, T, D, H = 4, 2048, 1024, 16
Dh = D // H                    # 64
NCORES = 8
F = D // 2                     # 512 features (8 heads) per core
KD = D // 128                  # 8 contraction tiles for projections
PAIRS = F // 128               # 4 head-pair feature tiles
NKT = T // 128                 # 16 key/value 128-blocks
NTC = T // 512                 # 4 query chunks of 512
HL = H // 2                    # 8 local heads

F32 = mybir.dt.float32
BF16 = mybir.dt.bfloat16
EXP = mybir.ActivationFunctionType.Exp

_NC_CACHE = None


def build_nc():
    nc = bacc.Bacc(None, target_bir_lowering=False, debug=False)

    xT = nc.dram_tensor("xT", [D, T], BF16, kind="ExternalInput")
    wqT = nc.dram_tensor("wqT", [D, F], BF16, kind="ExternalInput")
    wkT = nc.dram_tensor("wkT", [D, F], BF16, kind="ExternalInput")
    wvT = nc.dram_tensor("wvT", [D, F], BF16, kind="ExternalInput")
    woT = nc.dram_tensor("woT", [F, D], BF16, kind="ExternalInput")
    tri = nc.dram_tensor("tri", [128, 128], BF16, kind="ExternalInput")
    ident = nc.dram_tensor("ident", [128, 128], BF16, kind="ExternalInput")
    outT = nc.dram_tensor("outT", [D, T], BF16, kind="ExternalOutput")

    with TileContext(nc) as tc:
        with (
            tc.tile_pool(name="persist", bufs=1) as persist,
            tc.tile_pool(name="ptpool", bufs=22) as ptpool,
            tc.tile_pool(name="ybpool", bufs=2) as ybpool,
            tc.tile_pool(name="recpool", bufs=4) as recpool,
            tc.tile_pool(name="ostage", bufs=4) as ostage,
            tc.tile_pool(name="pjo", bufs=2, space="PSUM") as pjo,
            tc.tile_pool(name="sps", bufs=2, space="PSUM") as sps,
        ):
            xt = persist.tile([128, KD, T], BF16)
            kTt = persist.tile([128, PAIRS, T], BF16)
            qTt = persist.tile([128, PAIRS, T], BF16)
            vo = persist.tile([128, NKT, HL, Dh + 1], BF16)
            yTt = persist.tile([128, PAIRS, T], BF16)
            wo = persist.tile([128, PAIRS, D], BF16)
            wkt = persist.tile([128, KD, F], BF16)
            wqt = persist.tile([128, KD, F], BF16)
            wvt = persist.tile([128, KD, F], BF16)
            trit = persist.tile([128, 128], BF16)
            idt = persist.tile([128, 128], BF16)
            onesc = persist.tile([128, 1], BF16)

            nc.vector.memset(onesc[:], 1.0)
            nc.vector.tensor_copy(
                vo[:, :, :, Dh : Dh + 1], onesc.broadcast_to([128, NKT, HL, 1])
            )

            xTr = xT.rearrange("(k p) t -> p k t", p=128)
            wkr = wkT.rearrange("(k p) f -> p k f", p=128)
            wqr = wqT.rearrange("(k p) f -> p k f", p=128)
            # DMA order follows the consumption order of the software
            # pipeline: just enough of wk/x/wq to start k(0,0)/q(0,0), then
            # the rest in first-use order (transfers serialize on the DMA
            # engines, so order = arrival time).
            nc.sync.dma_start(wkt[:, :, 0:128], wkr[:, :, 0:128])
            nc.sync.dma_start(xt[:, 0:2, 0:512], xTr[:, 0:2, 0:512])
            nc.sync.dma_start(xt[:, 2:4, 0:512], xTr[:, 2:4, 0:512])
            nc.sync.dma_start(wqt[:, :, 0:128], wqr[:, :, 0:128])
            nc.sync.dma_start(xt[:, 4:6, 0:512], xTr[:, 4:6, 0:512])
            nc.sync.dma_start(xt[:, 6:8, 0:512], xTr[:, 6:8, 0:512])
            nc.sync.dma_start(wkt[:, :, 128:512], wkr[:, :, 128:512])
            nc.sync.dma_start(wqt[:, :, 128:512], wqr[:, :, 128:512])
            nc.sync.dma_start(tri_:=trit[:], tri[:])
            nc.sync.dma_start(wvt[:], wvT.rearrange("(k p) f -> p k f", p=128))
            nc.sync.dma_start(idt[:], ident[:])
            nc.sync.dma_start(xt[:, :, 512:1024], xTr[:, :, 512:1024])
            nc.sync.dma_start(xt[:, :, 1024:1536], xTr[:, :, 1024:1536])
            nc.sync.dma_start(xt[:, :, 1536:2048], xTr[:, :, 1536:2048])
            nc.sync.dma_start(wo[:], woT.rearrange("(k p) m -> p k m", p=128))

            def proj_qk(wt, dst, c, t):
                """dst[:, t, 512c:512c+512] = (W^T x)[128 f-rows of pair t]."""
                ps = pjo.tile([128, 512], F32, name="qk_ps", tag="pjo")
                for k in range(KD):
                    nc.tensor.matmul(
                        ps[:],
                        wt[:, k, 128 * t : 128 * t + 128],
                        xt[:, k, 512 * c : 512 * c + 512],
                        start=(k == 0),
                        stop=(k == KD - 1),
                    )
                nc.vector.tensor_copy(dst[:, t, 512 * c : 512 * c + 512], ps[:])

            def proj_v(tb):
                """vo[:, tb, :, 0:64] = x[128 toks of tb] @ W_v (head-major)."""
                ps = pjo.tile([128, 512], F32, name="v_ps", tag="pjo")
                # one accumulation group at a time per psum bank
                for q4 in range(2):
                    for k in range(KD):
                        nc.tensor.matmul(
                            ps[:, 256 * q4 : 256 * q4 + 256],
                            xt[:, k, 128 * tb : 128 * tb + 128],
                            wvt[:, k, 256 * q4 : 256 * q4 + 256],
                            start=(k == 0),
                            stop=(k == KD - 1),
                        )
                nc.vector.tensor_copy(
                    vo[:, tb, :, 0:Dh], ps.rearrange("p (h d) -> p h d", d=Dh)
                )

            def s_group(j, h, kbs, pts):
                """Scores for up to 3 full key blocks OR the 4 diagonal
                blocks of chunk j, gap-free-packed into one [128, 1536]
                psum tile (each block inside a single 512-col bank) so a
                single exp instruction covers the group.  pts[h][kb]
                records (pt_tile, base - col0) for pv_q's addressing.
                """
                t, s = h // 2, h % 2
                rows = slice(64 * s, 64 * s + 64)
                ps = sps.tile([128, 1536], F32, name="s_ps")
                base = 0
                placements = []
                for kb in kbs:
                    d = kb - 4 * j
                    col0 = 128 * d if d > 0 else 0
                    placements.append((kb, base, col0))
                    base += 512 - col0
                total = base
                for kb, bb, col0 in placements:
                    nc.tensor.matmul(
                        ps[:, bb : bb + 512 - col0],
                        kTt[rows, t, 128 * kb : 128 * kb + 128],
                        qTt[rows, t, 512 * j + col0 : 512 * j + 512],
                        start=True,
                        stop=True,
                    )
                pt = ptpool.tile([128, 1536], BF16, name="pt")
                nc.scalar.activation(
                    pt[:, 0:total], ps[:, 0:total], EXP,
                    scale=float(Dh) ** -0.5,
                )
                for kb, bb, col0 in placements:
                    if kb >= 4 * j:
                        # on gpsimd: keeps the DVE queue free of exp-waiting
                        # work (recip/norm must not queue behind masks)
                        nc.gpsimd.tensor_mul(
                            pt[:, bb : bb + 128],
                            pt[:, bb : bb + 128],
                            trit[:],
                        )
                    pts.setdefault(h, {})[kb] = (pt, bb - col0)

            def s_unit(j, h, pts):
                full = list(range(4 * j))
                for i in range(0, len(full), 3):
                    s_group(j, h, full[i : i + 3], pts)
                # diagonal quad ordered so widths (512,384,128,256) pack
                # the banks as 512 | 384+128 | 256: gap- and crossing-free
                s_group(j, h, [4 * j, 4 * j + 1, 4 * j + 3, 4 * j + 2], pts)

            def pv_q(j, h, b, pts, ypss):
                """P@V accumulation for q-block b of (chunk j, head h)."""
                if h not in ypss:
                    ypss[h] = pjo.tile(
                        [128, 4, Dh + 1], F32, name="y_ps", tag="pjo"
                    )
                yps = ypss[h]
                qb = 4 * j + b
                for kb in range(qb + 1):
                    pt, off0 = pts[h][kb]
                    nc.tensor.matmul(
                        yps[:, b, :],
                        pt[:, off0 + 128 * b : off0 + 128 * b + 128],
                        vo[:, kb, h, :],
                        start=(kb == 0),
                        stop=(kb == qb),
                    )

            def pv_fin(j, h, ypss, yb):
                """reciprocal of denominators + normalized write to yb."""
                yps = ypss.pop(h)
                rec = recpool.tile([128, 4, 1], F32, name="rec")
                nc.vector.reciprocal(rec[:], yps[:, :, Dh : Dh + 1])
                nc.vector.tensor_mul(
                    yb[:, :, Dh * h : Dh * h + Dh],
                    yps[:, :, 0:Dh],
                    rec.broadcast_to([128, 4, Dh]),
                )

            def pv_unit(j, h, pts, yb, ypss=None):
                if ypss is None:
                    ypss = {}
                for b in range(4):
                    pv_q(j, h, b, pts, ypss)
                pv_fin(j, h, ypss, yb)

            def transpose_ft(j, ft, yb):
                """yTt[:, ft, chunk j] = yb[:, :, ftile].T (bf16 PE blocks).

                Depends only on heads 2ft and 2ft+1, so it can run right
                after their PV units instead of at the chunk tail.
                """
                tps = pjo.tile([128, 4, 128], BF16, name="t_ps", tag="pjo")
                for b in range(4):
                    nc.tensor.matmul(
                        tps[:, b, :],
                        yb[:, b, 128 * ft : 128 * ft + 128],
                        idt[:],
                        is_transpose=True,
                    )
                nc.vector.tensor_copy(
                    yTt[:, ft, 512 * j : 512 * j + 512],
                    tps.rearrange("p b q -> p (b q)"),
                )

            def oproj(j, m):
                ps = pjo.tile([128, 512], F32, name="o_ps", tag="pjo")
                for kf in range(PAIRS):
                    nc.tensor.matmul(
                        ps[:],
                        wo[:, kf, 128 * m : 128 * m + 128],
                        yTt[:, kf, 512 * j : 512 * j + 512],
                        start=(kf == 0),
                        stop=(kf == PAIRS - 1),
                    )
                st = ostage.tile([128, 512], BF16, name="o_st", tag="o_st")
                nc.vector.tensor_copy(st[:], ps[:])
                nc.sync.dma_start(
                    outT[128 * m : 128 * m + 128, 512 * j : 512 * j + 512],
                    st[:],
                )

            # ---- schedule ----------------------------------------------
            # Per chunk j: S (scores+exp+mask) and P (P@V+normalize) units
            # pipelined with skew 2 so exp(h) overlaps later matmuls, and
            # projection/oproj filler woven between units to keep the PE
            # busy while the Activation engine chews on exp.
            def chunk_stream(j, fillers, pts=None, first_s=0, extra_s=(), back=False):
                """Unit steps with fillers interleaved evenly (chunks >= 1).

                pts/first_s: S units of this chunk already issued by the
                previous chunk's stream (pulled heads).  extra_s: S units of
                the NEXT chunk to issue at this chunk's tail, as
                [(h, pts_next)] entries.
                """
                if pts is None:
                    pts = {}
                yb = ybpool.tile([128, 4, F], BF16, name="yb", tag="yb")
                order = []
                for h in range(first_s, 8):
                    if h >= 2:
                        order.append(("p", h - 2))
                        if (h - 2) % 2 == 1:
                            order.append(("t", (h - 2) // 2))
                    order.append(("s", h))
                order += [("p", 6), ("p", 7), ("t", 3)]
                # weave pulled next-chunk S units in before the last PV
                # units so the Activation engine is fed across the chunk
                # boundary instead of only at the very end
                for i, (h_nxt, pts_nxt) in enumerate(extra_s):
                    order.insert(len(order) - 3 + i, ("x", (h_nxt, pts_nxt)))
                steps = []
                for kind, h in order:
                    if kind == "s":
                        steps.append(lambda h=h: s_unit(j, h, pts))
                    elif kind == "p":
                        steps.append(lambda h=h: pv_unit(j, h, pts, yb))
                    elif kind == "t":
                        steps.append(lambda h=h: transpose_ft(j, h, yb))
                    else:
                        steps.append(
                            lambda h=h[0], d=h[1]: s_unit(j + 1, h, d)
                        )
                # weave fillers between steps
                nf, ns = len(fillers), len(steps)
                out = []
                fi = 0
                for si, st_ in enumerate(steps):
                    out.append(st_)
                    if back:
                        # back-loaded: reserve fillers for the tail units
                        want = int(round(nf * ((si + 1) / ns) ** 2))
                    else:
                        want = int(round((si + 1) * nf / ns))
                    while fi < min(want, nf):
                        out.append(fillers[fi])
                        fi += 1
                while fi < nf:
                    out.append(fillers[fi])
                    fi += 1
                return out

            # ---- chunk 0 (hand-ordered) --------------------------------
            # k/q pair tiles and v blocks are issued before the S/P units
            # that consume them; k/q of chunk 1 and v(4..7) (chunk 1's PV
            # inputs) ride along as PE filler under chunk 0's exp.
            pts0 = {}
            yb0 = ybpool.tile([128, 4, F], BF16, name="yb0", tag="yb")
            proj_qk(wkt, kTt, 0, 0)
            proj_qk(wqt, qTt, 0, 0)
            ops0 = [
                lambda: s_unit(0, 0, pts0),
                lambda: s_unit(0, 1, pts0),
                lambda: proj_qk(wkt, kTt, 0, 1),
                lambda: proj_qk(wqt, qTt, 0, 1),
                lambda: s_unit(0, 2, pts0),
                lambda: s_unit(0, 3, pts0),
                lambda: proj_qk(wkt, kTt, 0, 2),
                lambda: proj_qk(wqt, qTt, 0, 2),
                lambda: s_unit(0, 4, pts0),
                lambda: proj_v(0),
                lambda: proj_v(1),
                lambda: proj_v(2),
                lambda: proj_v(3),
                lambda: pv_unit(0, 0, pts0, yb0),
                lambda: proj_qk(wkt, kTt, 0, 3),
                lambda: proj_qk(wqt, qTt, 0, 3),
                lambda: s_unit(0, 5, pts0),
                lambda: pv_unit(0, 1, pts0, yb0),
                lambda: transpose_ft(0, 0, yb0),
                lambda: proj_qk(wkt, kTt, 1, 0),
                lambda: proj_qk(wqt, qTt, 1, 0),
                lambda: s_unit(0, 6, pts0),
                lambda: pv_unit(0, 2, pts0, yb0),
                lambda: proj_qk(wkt, kTt, 1, 1),
                lambda: proj_qk(wqt, qTt, 1, 1),
                lambda: s_unit(0, 7, pts0),
                lambda: pv_unit(0, 3, pts0, yb0),
                lambda: transpose_ft(0, 1, yb0),
                lambda: proj_qk(wkt, kTt, 1, 2),
                lambda: proj_qk(wqt, qTt, 1, 2),
                lambda: pv_unit(0, 4, pts0, yb0),
                lambda: proj_qk(wkt, kTt, 1, 3),
                lambda: proj_qk(wqt, qTt, 1, 3),
                lambda: pv_unit(0, 5, pts0, yb0),
                lambda: transpose_ft(0, 2, yb0),
                lambda: proj_v(4),
                lambda: proj_v(5),
                lambda: pv_unit(0, 6, pts0, yb0),
                lambda: proj_v(6),
                lambda: proj_v(7),
                lambda: pv_unit(0, 7, pts0, yb0),
                lambda: transpose_ft(0, 3, yb0),
            ]

            f1 = []
            for t in range(PAIRS):
                f1.append(lambda t=t: proj_qk(wkt, kTt, 2, t))
                f1.append(lambda t=t: proj_qk(wqt, qTt, 2, t))
            for tb in range(8, 12):
                f1.append(lambda tb=tb: proj_v(tb))

            # chunk 2 fillers: k/q of chunk 3 first (the pulled chunk-3 S
            # units at the end of chunk 2 need them), then v(12..15) and
            # oproj(0); the first two chunk-3 S units are woven at chunk
            # 2's tail to pre-feed the Activation engine.
            pts2 = {}
            pts3 = {}
            f2 = []
            for t in range(PAIRS):
                f2.append(lambda t=t: proj_qk(wkt, kTt, 3, t))
                f2.append(lambda t=t: proj_qk(wqt, qTt, 3, t))
            for tb in range(12, 16):
                f2.append(lambda tb=tb: proj_v(tb))
            for m in range(4):
                f2.append(lambda m=m: oproj(0, m))

            f3 = [lambda j=j, m=m: oproj(j, m)
                  for (j, m) in [(0, m) for m in range(4, 8)]
                  + [(1, m) for m in range(8)] + [(2, m) for m in range(8)]]

            for op in ops0:
                op()
            for op in chunk_stream(1, f1):
                op()
            for op in chunk_stream(2, f2, extra_s=[(0, pts3), (1, pts3)]):
                op()
            for op in chunk_stream(3, f3, pts=pts3, first_s=2):
                op()
            for m in range(8):
                oproj(3, m)

    nc.finalize()
    return nc


def make_in_maps(x, W_q, W_k, W_v, W_o):
    bf = ml_dtypes.bfloat16
    tri = np.triu(np.ones((128, 128), dtype=np.float32)).astype(bf)
    idm = np.eye(128, dtype=np.float32).astype(bf)
    in_maps = []
    for c in range(NCORES):
        b, g = c // 2, c % 2
        fs = slice(F * g, F * g + F)
        in_maps.append(
            {
                "xT": np.ascontiguousarray(x[b].T).astype(bf),
                "wqT": np.ascontiguousarray(W_q[fs, :].T).astype(bf),
                "wkT": np.ascontiguousarray(W_k[fs, :].T).astype(bf),
                "wvT": np.ascontiguousarray(W_v[fs, :].T).astype(bf),
                "woT": np.ascontiguousarray(W_o[:, fs].T).astype(bf),
                "tri": tri,
                "ident": idm,
            }
        )
    return in_maps


def kernel(x, W_q, W_k, W_v, W_o, b_o):
    global _NC_CACHE
    x = np.asarray(x, dtype=np.float32)
    W_q = np.asarray(W_q, dtype=np.float32)
    W_k = np.asarray(W_k, dtype=np.float32)
    W_v = np.asarray(W_v, dtype=np.float32)
    W_o = np.asarray(W_o, dtype=np.float32)
    b_o = np.asarray(b_o, dtype=np.float32)

    if _NC_CACHE is None:
        _NC_CACHE = build_nc()
    nc = _NC_CACHE

    in_maps = make_in_maps(x, W_q, W_k, W_v, W_o)
    res = run_bass_kernel_spmd(nc, in_maps, core_ids=list(range(NCORES)))

    out = np.empty((B, T, D), dtype=np.float32)
    for b in range(B):
        acc = res.results[2 * b]["outT"].astype(np.float32) + res.results[
            2 * b + 1
        ]["outT"].astype(np.float32)
        out[b] = acc.T + b_o
    return out


if __name__ == "__main__":
    rng = np.random.default_rng(0)
    inputs = {
        "x": rng.standard_normal((B, T, D), dtype=np.float32),
        "W_q": rng.standard_normal((D, D), dtype=np.float32) / 32,
        "W_k": rng.standard_normal((D, D), dtype=np.float32) / 32,
        "W_v": rng.standard_normal((D, D), dtype=np.float32) / 32,
        "W_o": rng.standard_normal((D, D), dtype=np.float32) / 32,
        "b_o": rng.standard_normal((D,), dtype=np.float32) * 0.02,
    }
    out = kernel(**inputs)
    print("ran ok", out.shape, out.dtype)


# revision 66
# speedup vs baseline: 1.0085x; 1.0005x over previous
"""Causal self-attention (B=4, T=2048, D=1024, H=16) on 8 trn2 NeuronCores.

Sharding: core c handles batch b=c//2 and head-group g=c%2 (8 heads, 512
features). Each core computes q/k/v projections for its feature slice, causal
attention for its 8 heads, and a partial output projection (row-parallel W_o).
The host sums the two partial outputs per batch and adds b_o.

All matmuls run in bf16 (inputs cast on host; fp32 psum accumulation).
Design notes (cost model: matmul cost = out free size x 1 cyc/row @2.4GHz):
- P@V is computed as out[q_block(128), Dh+1] so each matmul pays only 65
  columns instead of 512 -> P@V costs 8840 cols/head vs 17408.  The ones
  column appended to v gives the softmax denominator in the same matmuls.
- y comes out [q, f]; a PE transpose (128x128 bf16 blocks via identity)
  restores y^T[f, q] for the row-parallel output projection.
- softmax exp runs on the Activation engine (the #2 engine at ~150us);
  projections are software-pipelined into the attention chunks so the PE
  never waits on exp.
- per-(q-block) denominators land in psum cols [*, b, 64]; one DVE
  reciprocal + one broadcast multiply per (chunk, head) normalizes y.
"""
import sys

sys.path.insert(0, "/opt/trn_rl_repo")

import numpy as np
import ml_dtypes

import concourse.bacc as bacc
import concourse.mybir as mybir
from concourse.tile import TileContext
from concourse.bass_utils import run_bass_kernel_spmd

B, T, D, H = 4, 2048, 1024, 16
Dh = D // H                    # 64
NCORES = 8
F = D // 2                     # 512 features (8 heads) per core
KD = D // 128                  # 8 contraction tiles for projections
PAIRS = F // 128               # 4 head-pair feature tiles
NKT = T // 128                 # 16 key/value 128-blocks
NTC = T // 512                 # 4 query chunks of 512
HL = H // 2                    # 8 local heads

F32 = mybir.dt.float32
BF16 = mybir.dt.bfloat16
EXP = mybir.ActivationFunctionType.Exp

_NC_CACHE = None


def build_nc():
    nc = bacc.Bacc(None, target_bir_lowering=False, debug=False)

    xT = nc.dram_tensor("xT", [D, T], BF16, kind="ExternalInput")
    wqT = nc.dram_tensor("wqT", [D, F], BF16, kind="ExternalInput")
    wkT = nc.dram_tensor("wkT", [D, F], BF16, kind="ExternalInput")
    wvT = nc.dram_tensor("wvT", [D, F], BF16, kind="ExternalInput")
    woT = nc.dram_tensor("woT", [F, D], BF16, kind="ExternalInput")
    tri = nc.dram_tensor("tri", [128, 128], BF16, kind="ExternalInput")
    ident = nc.dram_tensor("ident", [128, 128], BF16, kind="ExternalInput")
    outT = nc.dram_tensor("outT", [D, T], BF16, kind="ExternalOutput")

    with TileContext(nc) as tc:
        with (
            tc.tile_pool(name="persist", bufs=1) as persist,
            tc.tile_pool(name="ptpool", bufs=22) as ptpool,
            tc.tile_pool(name="ybpool", bufs=2) as ybpool,
            tc.tile_pool(name="recpool", bufs=4) as recpool,
            tc.tile_pool(name="ostage", bufs=4) as ostage,
            tc.tile_pool(name="pjo", bufs=2, space="PSUM") as pjo,
            tc.tile_pool(name="sps", bufs=2, space="PSUM") as sps,
        ):
            xt = persist.tile([128, KD, T], BF16)
            kTt = persist.tile([128, PAIRS, T], BF16)
            qTt = persist.tile([128, PAIRS, T], BF16)
            vo = persist.tile([128, NKT, HL, Dh + 1], BF16)
            yTt = persist.tile([128, PAIRS, T], BF16)
            wo = persist.tile([128, PAIRS, D], BF16)
            wkt = persist.tile([128, KD, F], BF16)
            wqt = persist.tile([128, KD, F], BF16)
            wvt = persist.tile([128, KD, F], BF16)
            trit = persist.tile([128, 128], BF16)
            idt = persist.tile([128, 128], BF16)
            onesc = persist.tile([128, 1], BF16)

            nc.vector.memset(onesc[:], 1.0)
            nc.vector.tensor_copy(
                vo[:, :, :, Dh : Dh + 1], onesc.broadcast_to([128, NKT, HL, 1])
            )

            xTr = xT.rearrange("(k p) t -> p k t", p=128)
            wkr = wkT.rearrange("(k p) f -> p k f", p=128)
            wqr = wqT.rearrange("(k p) f -> p k f", p=128)
            # DMA order follows the consumption order of the software
            # pipeline: just enough of wk/x/wq to start k(0,0)/q(0,0), then
            # the rest in first-use order (transfers serialize on the DMA
            # engines, so order = arrival time).
            nc.sync.dma_start(wkt[:, :, 0:128], wkr[:, :, 0:128])
            nc.sync.dma_start(xt[:, 0:2, 0:512], xTr[:, 0:2, 0:512])
            nc.sync.dma_start(xt[:, 2:4, 0:512], xTr[:, 2:4, 0:512])
            nc.sync.dma_start(xt[:, 4:6, 0:512], xTr[:, 4:6, 0:512])
            nc.sync.dma_start(xt[:, 6:8, 0:512], xTr[:, 6:8, 0:512])
            nc.sync.dma_start(wqt[:, :, 0:128], wqr[:, :, 0:128])
            nc.sync.dma_start(wkt[:, :, 128:512], wkr[:, :, 128:512])
            nc.sync.dma_start(wqt[:, :, 128:512], wqr[:, :, 128:512])
            nc.sync.dma_start(tri_:=trit[:], tri[:])
            nc.sync.dma_start(wvt[:], wvT.rearrange("(k p) f -> p k f", p=128))
            nc.sync.dma_start(idt[:], ident[:])
            nc.sync.dma_start(xt[:, :, 512:1024], xTr[:, :, 512:1024])
            nc.sync.dma_start(xt[:, :, 1024:1536], xTr[:, :, 1024:1536])
            nc.sync.dma_start(xt[:, :, 1536:2048], xTr[:, :, 1536:2048])
            nc.sync.dma_start(wo[:], woT.rearrange("(k p) m -> p k m", p=128))

            def proj_qk(wt, dst, c, t):
                """dst[:, t, 512c:512c+512] = (W^T x)[128 f-rows of pair t]."""
                ps = pjo.tile([128, 512], F32, name="qk_ps", tag="pjo")
                for k in range(KD):
                    nc.tensor.matmul(
                        ps[:],
                        wt[:, k, 128 * t : 128 * t + 128],
                        xt[:, k, 512 * c : 512 * c + 512],
                        start=(k == 0),
                        stop=(k == KD - 1),
                    )
                nc.vector.tensor_copy(dst[:, t, 512 * c : 512 * c + 512], ps[:])

            def proj_v(tb):
                """vo[:, tb, :, 0:64] = x[128 toks of tb] @ W_v (head-major)."""
                ps = pjo.tile([128, 512], F32, name="v_ps", tag="pjo")
                # one accumulation group at a time per psum bank
                for q4 in range(2):
                    for k in range(KD):
                        nc.tensor.matmul(
                            ps[:, 256 * q4 : 256 * q4 + 256],
                            xt[:, k, 128 * tb : 128 * tb + 128],
                            wvt[:, k, 256 * q4 : 256 * q4 + 256],
                            start=(k == 0),
                            stop=(k == KD - 1),
                        )
                nc.vector.tensor_copy(
                    vo[:, tb, :, 0:Dh], ps.rearrange("p (h d) -> p h d", d=Dh)
                )

            def s_group(j, h, kbs, pts):
                """Scores for up to 3 full key blocks OR the 4 diagonal
                blocks of chunk j, gap-free-packed into one [128, 1536]
                psum tile (each block inside a single 512-col bank) so a
                single exp instruction covers the group.  pts[h][kb]
                records (pt_tile, base - col0) for pv_q's addressing.
                """
                t, s = h // 2, h % 2
                rows = slice(64 * s, 64 * s + 64)
                ps = sps.tile([128, 1536], F32, name="s_ps")
                base = 0
                placements = []
                for kb in kbs:
                    d = kb - 4 * j
                    col0 = 128 * d if d > 0 else 0
                    placements.append((kb, base, col0))
                    base += 512 - col0
                total = base
                for kb, bb, col0 in placements:
                    nc.tensor.matmul(
                        ps[:, bb : bb + 512 - col0],
                        kTt[rows, t, 128 * kb : 128 * kb + 128],
                        qTt[rows, t, 512 * j + col0 : 512 * j + 512],
                        start=True,
                        stop=True,
                    )
                pt = ptpool.tile([128, 1536], BF16, name="pt")
                nc.scalar.activation(
                    pt[:, 0:total], ps[:, 0:total], EXP,
                    scale=float(Dh) ** -0.5,
                )
                for kb, bb, col0 in placements:
                    if kb >= 4 * j:
                        # on gpsimd: keeps the DVE queue free of exp-waiting
                        # work (recip/norm must not queue behind masks)
                        nc.gpsimd.tensor_mul(
                            pt[:, bb : bb + 128],
                            pt[:, bb : bb + 128],
                            trit[:],
                        )
                    pts.setdefault(h, {})[kb] = (pt, bb - col0)

            def s_unit(j, h, pts):
                full = list(range(4 * j))
                for i in range(0, len(full), 3):
                    s_group(j, h, full[i : i + 3], pts)
                # diagonal quad ordered so widths (512,384,128,256) pack
                # the banks as 512 | 384+128 | 256: gap- and crossing-free
                s_group(j, h, [4 * j, 4 * j + 1, 4 * j + 3, 4 * j + 2], pts)

            def pv_q(j, h, b, pts, ypss):
                """P@V accumulation for q-block b of (chunk j, head h)."""
                if h not in ypss:
                    ypss[h] = pjo.tile(
                        [128, 4, Dh + 1], F32, name="y_ps", tag="pjo"
                    )
                yps = ypss[h]
                qb = 4 * j + b
                for kb in range(qb + 1):
                    pt, off0 = pts[h][kb]
                    nc.tensor.matmul(
                        yps[:, b, :],
                        pt[:, off0 + 128 * b : off0 + 128 * b + 128],
                        vo[:, kb, h, :],
                        start=(kb == 0),
                        stop=(kb == qb),
                    )

            def pv_fin(j, h, ypss, yb):
                """reciprocal of denominators + normalized write to yb."""
                yps = ypss.pop(h)
                rec = recpool.tile([128, 4, 1], F32, name="rec")
                nc.vector.reciprocal(rec[:], yps[:, :, Dh : Dh + 1])
                nc.vector.tensor_mul(
                    yb[:, :, Dh * h : Dh * h + Dh],
                    yps[:, :, 0:Dh],
                    rec.broadcast_to([128, 4, Dh]),
                )

            def pv_unit(j, h, pts, yb, ypss=None):
                if ypss is None:
                    ypss = {}
                for b in range(4):
                    pv_q(j, h, b, pts, ypss)
                pv_fin(j, h, ypss, yb)

            def transpose_ft(j, ft, yb):
                """yTt[:, ft, chunk j] = yb[:, :, ftile].T (bf16 PE blocks).

                Depends only on heads 2ft and 2ft+1, so it can run right
                after their PV units instead of at the chunk tail.
                """
                tps = pjo.tile([128, 4, 128], BF16, name="t_ps", tag="pjo")
                for b in range(4):
                    nc.tensor.matmul(
                        tps[:, b, :],
                        yb[:, b, 128 * ft : 128 * ft + 128],
                        idt[:],
                        is_transpose=True,
                    )
                nc.vector.tensor_copy(
                    yTt[:, ft, 512 * j : 512 * j + 512],
                    tps.rearrange("p b q -> p (b q)"),
                )

            def oproj(j, m):
                ps = pjo.tile([128, 512], F32, name="o_ps", tag="pjo")
                for kf in range(PAIRS):
                    nc.tensor.matmul(
                        ps[:],
                        wo[:, kf, 128 * m : 128 * m + 128],
                        yTt[:, kf, 512 * j : 512 * j + 512],
                        start=(kf == 0),
                        stop=(kf == PAIRS - 1),
                    )
                st = ostage.tile([128, 512], BF16, name="o_st", tag="o_st")
                nc.vector.tensor_copy(st[:], ps[:])
                nc.sync.dma_start(
                    outT[128 * m : 128 * m + 128, 512 * j : 512 * j + 512],
                    st[:],
                )

            # ---- schedule ----------------------------------------------
            # Per chunk j: S (scores+exp+mask) and P (P@V+normalize) units
            # pipelined with skew 2 so exp(h) overlaps later matmuls, and
            # projection/oproj filler woven between units to keep the PE
            # busy while the Activation engine chews on exp.
            def chunk_stream(j, fillers, pts=None, first_s=0, extra_s=(), back=False):
                """Unit steps with fillers interleaved evenly (chunks >= 1).

                pts/first_s: S units of this chunk already issued by the
                previous chunk's stream (pulled heads).  extra_s: S units of
                the NEXT chunk to issue at this chunk's tail, as
                [(h, pts_next)] entries.
                """
                if pts is None:
                    pts = {}
                yb = ybpool.tile([128, 4, F], BF16, name="yb", tag="yb")
                order = []
                for h in range(first_s, 8):
                    if h >= 2:
                        order.append(("p", h - 2))
                        if (h - 2) % 2 == 1:
                            order.append(("t", (h - 2) // 2))
                    order.append(("s", h))
                order += [("p", 6), ("p", 7), ("t", 3)]
                # weave pulled next-chunk S units in before the last PV
                # units so the Activation engine is fed across the chunk
                # boundary instead of only at the very end
                for i, (h_nxt, pts_nxt) in enumerate(extra_s):
                    order.insert(len(order) - 3 + i, ("x", (h_nxt, pts_nxt)))
                steps = []
                for kind, h in order:
                    if kind == "s":
                        steps.append(lambda h=h: s_unit(j, h, pts))
                    elif kind == "p":
                        steps.append(lambda h=h: pv_unit(j, h, pts, yb))
                    elif kind == "t":
                        steps.append(lambda h=h: transpose_ft(j, h, yb))
                    else:
                        steps.append(
                            lambda h=h[0], d=h[1]: s_unit(j + 1, h, d)
                        )
                # weave fillers between steps
                nf, ns = len(fillers), len(steps)
                out = []
                fi = 0
                for si, st_ in enumerate(steps):
                    out.append(st_)
                    if back:
                        # back-loaded: reserve fillers for the tail units
                        want = int(round(nf * ((si + 1) / ns) ** 2))
                    else:
                        want = int(round((si + 1) * nf / ns))
                    while fi < min(want, nf):
                        out.append(fillers[fi])
                        fi += 1
                while fi < nf:
                    out.append(fillers[fi])
                    fi += 1
                return out

            # ---- chunk 0 (hand-ordered) --------------------------------
            # k/q pair tiles and v blocks are issued before the S/P units
            # that consume them; k/q of chunk 1 and v(4..7) (chunk 1's PV
            # inputs) ride along as PE filler under chunk 0's exp.
            pts0 = {}
            yb0 = ybpool.tile([128, 4, F], BF16, name="yb0", tag="yb")
            proj_qk(wkt, kTt, 0, 0)
            proj_qk(wqt, qTt, 0, 0)
            ops0 = [
                lambda: s_unit(0, 0, pts0),
                lambda: s_unit(0, 1, pts0),
                lambda: proj_qk(wkt, kTt, 0, 1),
                lambda: proj_qk(wqt, qTt, 0, 1),
                lambda: s_unit(0, 2, pts0),
                lambda: s_unit(0, 3, pts0),
                lambda: proj_qk(wkt, kTt, 0, 2),
                lambda: proj_qk(wqt, qTt, 0, 2),
                lambda: s_unit(0, 4, pts0),
                lambda: proj_v(0),
                lambda: proj_v(1),
                lambda: proj_v(2),
                lambda: proj_v(3),
                lambda: pv_unit(0, 0, pts0, yb0),
                lambda: proj_qk(wkt, kTt, 0, 3),
                lambda: proj_qk(wqt, qTt, 0, 3),
                lambda: s_unit(0, 5, pts0),
                lambda: pv_unit(0, 1, pts0, yb0),
                lambda: transpose_ft(0, 0, yb0),
                lambda: proj_qk(wkt, kTt, 1, 0),
                lambda: proj_qk(wqt, qTt, 1, 0),
                lambda: s_unit(0, 6, pts0),
                lambda: pv_unit(0, 2, pts0, yb0),
                lambda: proj_qk(wkt, kTt, 1, 1),
                lambda: proj_qk(wqt, qTt, 1, 1),
                lambda: s_unit(0, 7, pts0),
                lambda: pv_unit(0, 3, pts0, yb0),
                lambda: transpose_ft(0, 1, yb0),
                lambda: proj_qk(wkt, kTt, 1, 2),
                lambda: proj_qk(wqt, qTt, 1, 2),
                lambda: pv_unit(0, 4, pts0, yb0),
                lambda: proj_qk(wkt, kTt, 1, 3),
                lambda: proj_qk(wqt, qTt, 1, 3),
                lambda: pv_unit(0, 5, pts0, yb0),
                lambda: transpose_ft(0, 2, yb0),
                lambda: proj_v(4),
                lambda: proj_v(5),
                lambda: pv_unit(0, 6, pts0, yb0),
                lambda: proj_v(6),
                lambda: proj_v(7),
                lambda: pv_unit(0, 7, pts0, yb0),
                lambda: transpose_ft(0, 3, yb0),
            ]

            f1 = []
            for t in range(PAIRS):
                f1.append(lambda t=t: proj_qk(wkt, kTt, 2, t))
                f1.append(lambda t=t: proj_qk(wqt, qTt, 2, t))
            for tb in range(8, 12):
                f1.append(lambda tb=tb: proj_v(tb))

            # chunk 2 fillers: k/q of chunk 3 first (the pulled chunk-3 S
            # units at the end of chunk 2 need them), then v(12..15) and
            # oproj(0); the first two chunk-3 S units are woven at chunk
            # 2's tail to pre-feed the Activation engine.
            pts2 = {}
            pts3 = {}
            f2 = []
            for t in range(PAIRS):
                f2.append(lambda t=t: proj_qk(wkt, kTt, 3, t))
                f2.append(lambda t=t: proj_qk(wqt, qTt, 3, t))
            for tb in range(12, 16):
                f2.append(lambda tb=tb: proj_v(tb))
            for m in range(4):
                f2.append(lambda m=m: oproj(0, m))

            f3 = [lambda j=j, m=m: oproj(j, m)
                  for (j, m) in [(0, m) for m in range(4, 8)]
                  + [(1, m) for m in range(8)] + [(2, m) for m in range(8)]]

            for op in ops0:
                op()
            for op in chunk_stream(1, f1):
                op()
            for op in chunk_stream(2, f2, extra_s=[(0, pts3), (1, pts3)]):
                op()
            for op in chunk_stream(3, f3, pts=pts3, first_s=2):
                op()
            for m in range(8):
                oproj(3, m)

    nc.finalize()
    return nc


def make_in_maps(x, W_q, W_k, W_v, W_o):
    bf = ml_dtypes.bfloat16
    tri = np.triu(np.ones((128, 128), dtype=np.float32)).astype(bf)
    idm = np.eye(128, dtype=np.float32).astype(bf)
    in_maps = []
    for c in range(NCORES):
        b, g = c // 2, c % 2
        fs = slice(F * g, F * g + F)
        in_maps.append(
            {
                "xT": np.ascontiguousarray(x[b].T).astype(bf),
                "wqT": np.ascontiguousarray(W_q[fs, :].T).astype(bf),
                "wkT": np.ascontiguousarray(W_k[fs, :].T).astype(bf),
                "wvT": np.ascontiguousarray(W_v[fs, :].T).astype(bf),
                "woT": np.ascontiguousarray(W_o[:, fs].T).astype(bf),
                "tri": tri,
                "ident": idm,
            }
        )
    return in_maps


def kernel(x, W_q, W_k, W_v, W_o, b_o):
    global _NC_CACHE
    x = np.asarray(x, dtype=np.float32)
    W_q = np.asarray(W_q, dtype=np.float32)
    W_k = np.asarray(W_k, dtype=np.float32)
    W_v = np.asarray(W_v, dtype=np.float32)
    W_o = np.asarray(W_o, dtype=np.float32)
    b_o = np.asarray(b_o, dtype=np.float32)

    if _NC_CACHE is None:
        _NC_CACHE = build_nc()
    nc = _NC_CACHE

    in_maps = make_in_maps(x, W_q, W_k, W_v, W_o)
    res = run_bass_kernel_spmd(nc, in_maps, core_ids=list(range(NCORES)))

    out = np.empty((B, T, D), dtype=np.float32)
    for b in range(B):
        acc = res.results[2 * b]["outT"].astype(np.float32) + res.results[
            2 * b + 1
        ]["outT"].astype(np.float32)
        out[b] = acc.T + b_o
    return out


if __name__ == "__main__":
    rng = np.random.default_rng(0)
    inputs = {
        "x": rng.standard_normal((B, T, D), dtype=np.float32),
        "W_q": rng.standard_normal((D, D), dtype=np.float32) / 32,
        "W_k": rng.standard_normal((D, D), dtype=np.float32) / 32,
        "W_v": rng.standard_normal((D, D), dtype=np.float32) / 32,
        "W_o": rng.standard_normal((D, D), dtype=np.float32) / 32,
        "b_o": rng.standard_normal((D,), dtype=np.float32) * 0.02,
    }
    out = kernel(**inputs)
    print("ran ok", out.shape, out.dtype)


# revision 69
# speedup vs baseline: 1.0110x; 1.0026x over previous
"""Causal self-attention (B=4, T=2048, D=1024, H=16) on 8 trn2 NeuronCores.

Sharding: core c handles batch b=c//2 and head-group g=c%2 (8 heads, 512
features). Each core computes q/k/v projections for its feature slice, causal
attention for its 8 heads, and a partial output projection (row-parallel W_o).
The host sums the two partial outputs per batch and adds b_o.

All matmuls run in bf16 (inputs cast on host; fp32 psum accumulation).
Design notes (cost model: matmul cost = out free size x 1 cyc/row @2.4GHz):
- P@V is computed as out[q_block(128), Dh+1] so each matmul pays only 65
  columns instead of 512 -> P@V costs 8840 cols/head vs 17408.  The ones
  column appended to v gives the softmax denominator in the same matmuls.
- y comes out [q, f]; a PE transpose (128x128 bf16 blocks via identity)
  restores y^T[f, q] for the row-parallel output projection.
- softmax exp runs on the Activation engine (the #2 engine at ~150us);
  projections are software-pipelined into the attention chunks so the PE
  never waits on exp.
- per-(q-block) denominators land in psum cols [*, b, 64]; one DVE
  reciprocal + one broadcast multiply per (chunk, head) normalizes y.
"""
import sys

sys.path.insert(0, "/opt/trn_rl_repo")

import numpy as np
import ml_dtypes

import concourse.bacc as bacc
import concourse.mybir as mybir
from concourse.tile import TileContext
from concourse.bass_utils import run_bass_kernel_spmd

B, T, D, H = 4, 2048, 1024, 16
Dh = D // H                    # 64
NCORES = 8
F = D // 2                     # 512 features (8 heads) per core
KD = D // 128                  # 8 contraction tiles for projections
PAIRS = F // 128               # 4 head-pair feature tiles
NKT = T // 128                 # 16 key/value 128-blocks
NTC = T // 512                 # 4 query chunks of 512
HL = H // 2                    # 8 local heads

F32 = mybir.dt.float32
BF16 = mybir.dt.bfloat16
EXP = mybir.ActivationFunctionType.Exp

_NC_CACHE = None


def build_nc():
    nc = bacc.Bacc(None, target_bir_lowering=False, debug=False)

    xT = nc.dram_tensor("xT", [D, T], BF16, kind="ExternalInput")
    wqT = nc.dram_tensor("wqT", [D, F], BF16, kind="ExternalInput")
    wkT = nc.dram_tensor("wkT", [D, F], BF16, kind="ExternalInput")
    wvT = nc.dram_tensor("wvT", [D, F], BF16, kind="ExternalInput")
    woT = nc.dram_tensor("woT", [F, D], BF16, kind="ExternalInput")
    tri = nc.dram_tensor("tri", [128, 128], BF16, kind="ExternalInput")
    ident = nc.dram_tensor("ident", [128, 128], BF16, kind="ExternalInput")
    outT = nc.dram_tensor("outT", [D, T], BF16, kind="ExternalOutput")

    with TileContext(nc) as tc:
        with (
            tc.tile_pool(name="persist", bufs=1) as persist,
            tc.tile_pool(name="ptpool", bufs=22) as ptpool,
            tc.tile_pool(name="ybpool", bufs=2) as ybpool,
            tc.tile_pool(name="recpool", bufs=4) as recpool,
            tc.tile_pool(name="ostage", bufs=4) as ostage,
            tc.tile_pool(name="pjo", bufs=2, space="PSUM") as pjo,
            tc.tile_pool(name="sps", bufs=2, space="PSUM") as sps,
        ):
            xt = persist.tile([128, KD, T], BF16)
            kTt = persist.tile([128, PAIRS, T], BF16)
            qTt = persist.tile([128, PAIRS, T], BF16)
            vo = persist.tile([128, NKT, HL, Dh + 1], BF16)
            yTt = persist.tile([128, PAIRS, T], BF16)
            wo = persist.tile([128, PAIRS, D], BF16)
            wkt = persist.tile([128, KD, F], BF16)
            wqt = persist.tile([128, KD, F], BF16)
            wvt = persist.tile([128, KD, F], BF16)
            trit = persist.tile([128, 128], BF16)
            idt = persist.tile([128, 128], BF16)
            onesc = persist.tile([128, 1], BF16)

            nc.vector.memset(onesc[:], 1.0)
            nc.vector.tensor_copy(
                vo[:, :, :, Dh : Dh + 1], onesc.broadcast_to([128, NKT, HL, 1])
            )

            xTr = xT.rearrange("(k p) t -> p k t", p=128)
            wkr = wkT.rearrange("(k p) f -> p k f", p=128)
            wqr = wqT.rearrange("(k p) f -> p k f", p=128)
            # DMA order follows the consumption order of the software
            # pipeline: just enough of wk/x/wq to start k(0,0)/q(0,0), then
            # the rest in first-use order (transfers serialize on the DMA
            # engines, so order = arrival time).
            nc.sync.dma_start(wkt[:, :, 0:128], wkr[:, :, 0:128])
            nc.sync.dma_start(xt[:, 0:2, 0:512], xTr[:, 0:2, 0:512])
            nc.sync.dma_start(xt[:, 2:4, 0:512], xTr[:, 2:4, 0:512])
            nc.sync.dma_start(xt[:, 4:6, 0:512], xTr[:, 4:6, 0:512])
            nc.sync.dma_start(xt[:, 6:8, 0:512], xTr[:, 6:8, 0:512])
            nc.sync.dma_start(wqt[:, :, 0:128], wqr[:, :, 0:128])
            nc.sync.dma_start(wkt[:, :, 128:256], wkr[:, :, 128:256])
            nc.sync.dma_start(wqt[:, :, 128:256], wqr[:, :, 128:256])
            nc.sync.dma_start(wkt[:, :, 256:512], wkr[:, :, 256:512])
            nc.sync.dma_start(wqt[:, :, 256:512], wqr[:, :, 256:512])
            nc.sync.dma_start(tri_:=trit[:], tri[:])
            nc.sync.dma_start(wvt[:], wvT.rearrange("(k p) f -> p k f", p=128))
            nc.sync.dma_start(idt[:], ident[:])
            nc.sync.dma_start(xt[:, :, 512:1024], xTr[:, :, 512:1024])
            nc.sync.dma_start(xt[:, :, 1024:1536], xTr[:, :, 1024:1536])
            nc.sync.dma_start(xt[:, :, 1536:2048], xTr[:, :, 1536:2048])
            nc.sync.dma_start(wo[:], woT.rearrange("(k p) m -> p k m", p=128))

            def proj_qk(wt, dst, c, t):
                """dst[:, t, 512c:512c+512] = (W^T x)[128 f-rows of pair t]."""
                ps = pjo.tile([128, 512], F32, name="qk_ps", tag="pjo")
                for k in range(KD):
                    nc.tensor.matmul(
                        ps[:],
                        wt[:, k, 128 * t : 128 * t + 128],
                        xt[:, k, 512 * c : 512 * c + 512],
                        start=(k == 0),
                        stop=(k == KD - 1),
                    )
                nc.vector.tensor_copy(dst[:, t, 512 * c : 512 * c + 512], ps[:])

            def proj_v(tb):
                """vo[:, tb, :, 0:64] = x[128 toks of tb] @ W_v (head-major)."""
                ps = pjo.tile([128, 512], F32, name="v_ps", tag="pjo")
                # one accumulation group at a time per psum bank
                for q4 in range(2):
                    for k in range(KD):
                        nc.tensor.matmul(
                            ps[:, 256 * q4 : 256 * q4 + 256],
                            xt[:, k, 128 * tb : 128 * tb + 128],
                            wvt[:, k, 256 * q4 : 256 * q4 + 256],
                            start=(k == 0),
                            stop=(k == KD - 1),
                        )
                nc.vector.tensor_copy(
                    vo[:, tb, :, 0:Dh], ps.rearrange("p (h d) -> p h d", d=Dh)
                )

            def s_group(j, h, kbs, pts):
                """Scores for up to 3 full key blocks OR the 4 diagonal
                blocks of chunk j, gap-free-packed into one [128, 1536]
                psum tile (each block inside a single 512-col bank) so a
                single exp instruction covers the group.  pts[h][kb]
                records (pt_tile, base - col0) for pv_q's addressing.
                """
                t, s = h // 2, h % 2
                rows = slice(64 * s, 64 * s + 64)
                ps = sps.tile([128, 1536], F32, name="s_ps")
                base = 0
                placements = []
                for kb in kbs:
                    d = kb - 4 * j
                    col0 = 128 * d if d > 0 else 0
                    placements.append((kb, base, col0))
                    base += 512 - col0
                total = base
                for kb, bb, col0 in placements:
                    nc.tensor.matmul(
                        ps[:, bb : bb + 512 - col0],
                        kTt[rows, t, 128 * kb : 128 * kb + 128],
                        qTt[rows, t, 512 * j + col0 : 512 * j + 512],
                        start=True,
                        stop=True,
                    )
                pt = ptpool.tile([128, 1536], BF16, name="pt")
                nc.scalar.activation(
                    pt[:, 0:total], ps[:, 0:total], EXP,
                    scale=float(Dh) ** -0.5,
                )
                for kb, bb, col0 in placements:
                    if kb >= 4 * j:
                        # on gpsimd: keeps the DVE queue free of exp-waiting
                        # work (recip/norm must not queue behind masks)
                        nc.gpsimd.tensor_mul(
                            pt[:, bb : bb + 128],
                            pt[:, bb : bb + 128],
                            trit[:],
                        )
                    pts.setdefault(h, {})[kb] = (pt, bb - col0)

            def s_unit(j, h, pts):
                full = list(range(4 * j))
                for i in range(0, len(full), 3):
                    s_group(j, h, full[i : i + 3], pts)
                # diagonal quad ordered so widths (512,384,128,256) pack
                # the banks as 512 | 384+128 | 256: gap- and crossing-free
                s_group(j, h, [4 * j, 4 * j + 1, 4 * j + 3, 4 * j + 2], pts)

            def pv_q(j, h, b, pts, ypss):
                """P@V accumulation for q-block b of (chunk j, head h)."""
                if h not in ypss:
                    ypss[h] = pjo.tile(
                        [128, 4, Dh + 1], F32, name="y_ps", tag="pjo"
                    )
                yps = ypss[h]
                qb = 4 * j + b
                for kb in range(qb + 1):
                    pt, off0 = pts[h][kb]
                    nc.tensor.matmul(
                        yps[:, b, :],
                        pt[:, off0 + 128 * b : off0 + 128 * b + 128],
                        vo[:, kb, h, :],
                        start=(kb == 0),
                        stop=(kb == qb),
                    )

            def pv_fin(j, h, ypss, yb):
                """reciprocal of denominators + normalized write to yb."""
                yps = ypss.pop(h)
                rec = recpool.tile([128, 4, 1], F32, name="rec")
                nc.vector.reciprocal(rec[:], yps[:, :, Dh : Dh + 1])
                nc.vector.tensor_mul(
                    yb[:, :, Dh * h : Dh * h + Dh],
                    yps[:, :, 0:Dh],
                    rec.broadcast_to([128, 4, Dh]),
                )

            def pv_unit(j, h, pts, yb, ypss=None):
                if ypss is None:
                    ypss = {}
                for b in range(4):
                    pv_q(j, h, b, pts, ypss)
                pv_fin(j, h, ypss, yb)

            def transpose_ft(j, ft, yb):
                """yTt[:, ft, chunk j] = yb[:, :, ftile].T (bf16 PE blocks).

                Depends only on heads 2ft and 2ft+1, so it can run right
                after their PV units instead of at the chunk tail.
                """
                tps = pjo.tile([128, 4, 128], BF16, name="t_ps", tag="pjo")
                for b in range(4):
                    nc.tensor.matmul(
                        tps[:, b, :],
                        yb[:, b, 128 * ft : 128 * ft + 128],
                        idt[:],
                        is_transpose=True,
                    )
                nc.vector.tensor_copy(
                    yTt[:, ft, 512 * j : 512 * j + 512],
                    tps.rearrange("p b q -> p (b q)"),
                )

            def oproj(j, m):
                ps = pjo.tile([128, 512], F32, name="o_ps", tag="pjo")
                for kf in range(PAIRS):
                    nc.tensor.matmul(
                        ps[:],
                        wo[:, kf, 128 * m : 128 * m + 128],
                        yTt[:, kf, 512 * j : 512 * j + 512],
                        start=(kf == 0),
                        stop=(kf == PAIRS - 1),
                    )
                st = ostage.tile([128, 512], BF16, name="o_st", tag="o_st")
                nc.vector.tensor_copy(st[:], ps[:])
                nc.sync.dma_start(
                    outT[128 * m : 128 * m + 128, 512 * j : 512 * j + 512],
                    st[:],
                )

            # ---- schedule ----------------------------------------------
            # Per chunk j: S (scores+exp+mask) and P (P@V+normalize) units
            # pipelined with skew 2 so exp(h) overlaps later matmuls, and
            # projection/oproj filler woven between units to keep the PE
            # busy while the Activation engine chews on exp.
            def chunk_stream(j, fillers, pts=None, first_s=0, extra_s=(), back=False):
                """Unit steps with fillers interleaved evenly (chunks >= 1).

                pts/first_s: S units of this chunk already issued by the
                previous chunk's stream (pulled heads).  extra_s: S units of
                the NEXT chunk to issue at this chunk's tail, as
                [(h, pts_next)] entries.
                """
                if pts is None:
                    pts = {}
                yb = ybpool.tile([128, 4, F], BF16, name="yb", tag="yb")
                order = []
                for h in range(first_s, 8):
                    if h >= 2:
                        order.append(("p", h - 2))
                        if (h - 2) % 2 == 1:
                            order.append(("t", (h - 2) // 2))
                    order.append(("s", h))
                order += [("p", 6), ("p", 7), ("t", 3)]
                # weave pulled next-chunk S units in before the last PV
                # units so the Activation engine is fed across the chunk
                # boundary instead of only at the very end
                for i, (h_nxt, pts_nxt) in enumerate(extra_s):
                    order.insert(len(order) - 3 + i, ("x", (h_nxt, pts_nxt)))
                steps = []
                for kind, h in order:
                    if kind == "s":
                        steps.append(lambda h=h: s_unit(j, h, pts))
                    elif kind == "p":
                        steps.append(lambda h=h: pv_unit(j, h, pts, yb))
                    elif kind == "t":
                        steps.append(lambda h=h: transpose_ft(j, h, yb))
                    else:
                        steps.append(
                            lambda h=h[0], d=h[1]: s_unit(j + 1, h, d)
                        )
                # weave fillers between steps
                nf, ns = len(fillers), len(steps)
                out = []
                fi = 0
                for si, st_ in enumerate(steps):
                    out.append(st_)
                    if back:
                        # back-loaded: reserve fillers for the tail units
                        want = int(round(nf * ((si + 1) / ns) ** 2))
                    else:
                        want = int(round((si + 1) * nf / ns))
                    while fi < min(want, nf):
                        out.append(fillers[fi])
                        fi += 1
                while fi < nf:
                    out.append(fillers[fi])
                    fi += 1
                return out

            # ---- chunk 0 (hand-ordered) --------------------------------
            # k/q pair tiles and v blocks are issued before the S/P units
            # that consume them; k/q of chunk 1 and v(4..7) (chunk 1's PV
            # inputs) ride along as PE filler under chunk 0's exp.
            pts0 = {}
            yb0 = ybpool.tile([128, 4, F], BF16, name="yb0", tag="yb")
            proj_qk(wkt, kTt, 0, 0)
            proj_qk(wqt, qTt, 0, 0)
            ops0 = [
                lambda: s_unit(0, 0, pts0),
                lambda: s_unit(0, 1, pts0),
                lambda: proj_qk(wkt, kTt, 0, 1),
                lambda: proj_qk(wqt, qTt, 0, 1),
                lambda: s_unit(0, 2, pts0),
                lambda: s_unit(0, 3, pts0),
                lambda: proj_qk(wkt, kTt, 0, 2),
                lambda: proj_qk(wqt, qTt, 0, 2),
                lambda: s_unit(0, 4, pts0),
                lambda: proj_v(0),
                lambda: proj_v(1),
                lambda: proj_v(2),
                lambda: proj_v(3),
                lambda: pv_unit(0, 0, pts0, yb0),
                lambda: proj_qk(wkt, kTt, 0, 3),
                lambda: proj_qk(wqt, qTt, 0, 3),
                lambda: s_unit(0, 5, pts0),
                lambda: pv_unit(0, 1, pts0, yb0),
                lambda: transpose_ft(0, 0, yb0),
                lambda: proj_qk(wkt, kTt, 1, 0),
                lambda: proj_qk(wqt, qTt, 1, 0),
                lambda: s_unit(0, 6, pts0),
                lambda: pv_unit(0, 2, pts0, yb0),
                lambda: proj_qk(wkt, kTt, 1, 1),
                lambda: proj_qk(wqt, qTt, 1, 1),
                lambda: s_unit(0, 7, pts0),
                lambda: pv_unit(0, 3, pts0, yb0),
                lambda: transpose_ft(0, 1, yb0),
                lambda: proj_qk(wkt, kTt, 1, 2),
                lambda: proj_qk(wqt, qTt, 1, 2),
                lambda: pv_unit(0, 4, pts0, yb0),
                lambda: proj_qk(wkt, kTt, 1, 3),
                lambda: proj_qk(wqt, qTt, 1, 3),
                lambda: pv_unit(0, 5, pts0, yb0),
                lambda: transpose_ft(0, 2, yb0),
                lambda: proj_v(4),
                lambda: proj_v(5),
                lambda: pv_unit(0, 6, pts0, yb0),
                lambda: proj_v(6),
                lambda: proj_v(7),
                lambda: pv_unit(0, 7, pts0, yb0),
                lambda: transpose_ft(0, 3, yb0),
            ]

            f1 = []
            for t in range(PAIRS):
                f1.append(lambda t=t: proj_qk(wkt, kTt, 2, t))
                f1.append(lambda t=t: proj_qk(wqt, qTt, 2, t))
            for tb in range(8, 12):
                f1.append(lambda tb=tb: proj_v(tb))

            # chunk 2 fillers: k/q of chunk 3 first (the pulled chunk-3 S
            # units at the end of chunk 2 need them), then v(12..15) and
            # oproj(0); the first two chunk-3 S units are woven at chunk
            # 2's tail to pre-feed the Activation engine.
            pts2 = {}
            pts3 = {}
            f2 = []
            for t in range(PAIRS):
                f2.append(lambda t=t: proj_qk(wkt, kTt, 3, t))
                f2.append(lambda t=t: proj_qk(wqt, qTt, 3, t))
            for tb in range(12, 16):
                f2.append(lambda tb=tb: proj_v(tb))
            for m in range(4):
                f2.append(lambda m=m: oproj(0, m))

            f3 = [lambda j=j, m=m: oproj(j, m)
                  for (j, m) in [(0, m) for m in range(4, 8)]
                  + [(1, m) for m in range(8)] + [(2, m) for m in range(8)]]

            for op in ops0:
                op()
            for op in chunk_stream(1, f1):
                op()
            for op in chunk_stream(2, f2, extra_s=[(0, pts3), (1, pts3)]):
                op()
            for op in chunk_stream(3, f3, pts=pts3, first_s=2):
                op()
            for m in range(8):
                oproj(3, m)

    nc.finalize()
    return nc


def make_in_maps(x, W_q, W_k, W_v, W_o):
    bf = ml_dtypes.bfloat16
    tri = np.triu(np.ones((128, 128), dtype=np.float32)).astype(bf)
    idm = np.eye(128, dtype=np.float32).astype(bf)
    in_maps = []
    for c in range(NCORES):
        b, g = c // 2, c % 2
        fs = slice(F * g, F * g + F)
        in_maps.append(
            {
                "xT": np.ascontiguousarray(x[b].T).astype(bf),
                "wqT": np.ascontiguousarray(W_q[fs, :].T).astype(bf),
                "wkT": np.ascontiguousarray(W_k[fs, :].T).astype(bf),
                "wvT": np.ascontiguousarray(W_v[fs, :].T).astype(bf),
                "woT": np.ascontiguousarray(W_o[:, fs].T).astype(bf),
                "tri": tri,
                "ident": idm,
            }
        )
    return in_maps


def kernel(x, W_q, W_k, W_v, W_o, b_o):
    global _NC_CACHE
    x = np.asarray(x, dtype=np.float32)
    W_q = np.asarray(W_q, dtype=np.float32)
    W_k = np.asarray(W_k, dtype=np.float32)
    W_v = np.asarray(W_v, dtype=np.float32)
    W_o = np.asarray(W_o, dtype=np.float32)
    b_o = np.asarray(b_o, dtype=np.float32)

    if _NC_CACHE is None:
        _NC_CACHE = build_nc()
    nc = _NC_CACHE

    in_maps = make_in_maps(x, W_q, W_k, W_v, W_o)
    res = run_bass_kernel_spmd(nc, in_maps, core_ids=list(range(NCORES)))

    out = np.empty((B, T, D), dtype=np.float32)
    for b in range(B):
        acc = res.results[2 * b]["outT"].astype(np.float32) + res.results[
            2 * b + 1
        ]["outT"].astype(np.float32)
        out[b] = acc.T + b_o
    return out


if __name__ == "__main__":
    rng = np.random.default_rng(0)
    inputs = {
        "x": rng.standard_normal((B, T, D), dtype=np.float32),
        "W_q": rng.standard_normal((D, D), dtype=np.float32) / 32,
        "W_k": rng.standard_normal((D, D), dtype=np.float32) / 32,
        "W_v": rng.standard_normal((D, D), dtype=np.float32) / 32,
        "W_o": rng.standard_normal((D, D), dtype=np.float32) / 32,
        "b_o": rng.standard_normal((D,), dtype=np.float32) * 0.02,
    }
    out = kernel(**inputs)
    print("ran ok", out.shape, out.dtype)
